# revision 26
# baseline (speedup 1.0000x reference)
"""GAT message-passing kernel for Trainium2, 8 NeuronCores.

Problem (see harness reference): for each head h:
    Wh   = x @ W[h]                                  [B,N,F]
    e    = leaky_relu((Wh@a_src)[:,:,None] + (Wh@a_dst)[:,None,:], 0.2)
    att  = exp(where(adj>0, e, -9e15)) * big_w        [B,N,N]
    att /= clip(sum(att, axis=1), 1e-12)              (column L1 norm)
    out_h = elu(att @ Wh)
    out   = concat over heads                         [B,N,H*F]

big_w is bipartite: nonzero only on blocks (i<U, j>=U) [= weights.T] and
(i>=U, j<U) [= weights]. So att has only two 1024x1024 nonzero blocks.

Sharding: core c -> (b, block, head-pair) with b = c//4, blk = (c//2)%2,
hp = c%2.  Each core handles ONE bipartite block (its 1024 destination
rows i and 1024 source columns j) for TWO heads -> denominators are
core-local (each att column lives inside one block) and each core owns
1024 full output rows for its 2 heads.  No collectives, uniform SPMD.

All math runs in the transposed [j, i] layout.  The host pre-arranges
each core's shards so the device does ZERO transposes:
  - adjt: the core's adj block, transposed to [j, i] and row-tile packed
    to [128, 8*1024] (partition p, tile t, col i  <- adjT[t*128+p, i])
  - wq:   matching w values in the same [j, i] packed layout
  - xt:   x[b].T with columns ordered [i-range | j-range]
  - w2:   [128, 256] = W[h0] | W[h1],  av: [128,4] = a_src/a_dst pairs
  - w2t:  [128, 256] = W[h0].T | W[h1].T, for d = x @ (W a_dst)
Per-column exp factor cancellation: with z = s_i + d_j,
  exp(lrelu(z)) = max(e^z, e^az) = e^{d_j} * max(es_i, r_j * eas_i)
  (es = e^s, eas = e^{a s}, r = e^{(a-1)d}).  The e^{d_j} row factor
cancels against the denominator, so per head-tile the attention needs
just: m = (eas*r) max es (one DVE stt) and G = m*adjw with fused
row-sum -> den (one stt with accum_out).  Engine balance: head 0 and
half of head 1's tiles ride ACT (Prelu then Exp, bias=d column); the
rest use the DVE stt; adjw = adj*w builds on GpSimd (the only legal
Pool tensor_tensor: mult).  adj loads as a casting SWDGE DMA
(int32 -> bf16); x then w stream on the SP HWDGE ring (the ACT ring is
~3x slower -- params only); partition broadcasts are PE rank-1 outer
products (ones x row) because GpSimd blocks while SWDGE drains.
Scores use associativity: u = W @ a_dst via host-fed W^T, then
d[j] = x[j] . u -- 8 quad matmuls sharing the xtr stationary tile.
Output is accumulated transposed: outT[f,i] += whs[j,f]^T @ G[j,i]
with whs = Wh[j]/den[j] (Wh tiles matmul'd on demand, scaled from
PSUM), so matmuls are 512-wide; host un-transposes at gather.
elu(x) = max(x,0) + min(exp(x)-1, 0) with bf16 exp, stores per half.
"""

import threading
import numpy as np

B, N, FIN, F, H, U = 2, 2048, 128, 128, 4, 1024
P = 128
JT = U // P            # 8 tiles over the block's j axis
ALPHA = 0.2
CH = 2                 # v-tiles per DMA chunk (1MB chunks)
NCHUNK = JT // CH

TRACE = False          # set by test.py for profiling runs
LAST_EXEC_NS = None    # exec_time_ns of the last traced run
_BUILD_LOCK = threading.Lock()
_CACHE = {}


def _build_program():
    from concourse import bacc
    import concourse.mybir as mybir
    import concourse.tile as tile

    dt = mybir.dt
    Alu = mybir.AluOpType
    Act = mybir.ActivationFunctionType

    nc = bacc.Bacc("TRN2", target_bir_lowering=False, debug=False, num_devices=8)

    adjt = nc.dram_tensor("adjt", [P, JT * U], dt.int32, kind="ExternalInput")
    wq = nc.dram_tensor("wq", [P, JT * U], dt.float32, kind="ExternalInput")
    xt = nc.dram_tensor("xt", [P, N], dt.float32r, kind="ExternalInput")
    w2 = nc.dram_tensor("w2", [P, 2 * F], dt.float32r, kind="ExternalInput")
    av = nc.dram_tensor("av", [P, 4], dt.float32r, kind="ExternalInput")
    w2t = nc.dram_tensor("w2t", [P, 2 * F], dt.float32r, kind="ExternalInput")
    outh = nc.dram_tensor("outh", [2, P, U], dt.float32, kind="ExternalOutput")

    with tile.TileContext(nc) as tc:
        with (
            tc.tile_pool(name="persist", bufs=1) as persist,
            tc.tile_pool(name="adj_ch", bufs=3) as adj_pool,
            tc.tile_pool(name="w_ch", bufs=3) as w_pool,
            tc.tile_pool(name="adjw", bufs=3) as adjw_pool,
            tc.tile_pool(name="lr", bufs=2) as lr_pool,
            tc.tile_pool(name="ee", bufs=4) as e_pool,
            tc.tile_pool(name="gg", bufs=4) as g_pool,
            tc.tile_pool(name="whs", bufs=4) as whs_pool,
            tc.tile_pool(name="elu", bufs=4) as elu_pool,
            tc.tile_pool(name="ps_out", bufs=1, space="PSUM") as ps_out,
            tc.tile_pool(name="ps_a", bufs=2, space="PSUM") as ps_a,
        ):
            # ---------------- phase 0: params, xT, whT, scores
            w2r = persist.tile([P, 2 * F], dt.float32r)
            nc.scalar.dma_start(out=w2r, in_=w2[:, :])
            avr = persist.tile([P, 4], dt.float32r)
            nc.scalar.dma_start(out=avr, in_=av[:, :])
            w2tr = persist.tile([P, 2 * F], dt.float32r)
            nc.scalar.dma_start(out=w2tr, in_=w2t[:, :])
            xtr = persist.tile([P, N], dt.float32r)
            nc.sync.dma_start(out=xtr[:, 0:U], in_=xt[:, 0:U])
            nc.sync.dma_start(out=xtr[:, U:N], in_=xt[:, U:N])

            # bulk streams, issued up-front on otherwise-idle queues:
            # w on the SP HWDGE ring, adj via casting SWDGE (int32 -> bf16).
            # Subtile deps let per-v-tile consumers start as slices land.
            wsb = persist.tile([P, JT * U], dt.float32)
            asb = persist.tile([P, JT * U], dt.bfloat16)
            # w follows xt on the SP ring; small leading chunks so
            # adjw[0] unblocks early
            for lo, hi in ((0, 1), (1, 2), (2, 5), (5, 8)):
                sl = slice(lo * U, hi * U)
                nc.sync.dma_start(out=wsb[:, sl], in_=wq[:, sl])
            for lo, hi in ((0, 1), (1, 2), (2, 5), (5, 8)):
                sl = slice(lo * U, hi * U)
                nc.gpsimd.dma_start(out=asb[:, sl], in_=adjt[:, sl])

            ones_b = persist.tile([1, P], dt.bfloat16)
            nc.vector.memset(ones_b, 1.0)

            whT = [persist.tile([P, N], dt.float32r, name=f"whT{k}") for k in range(2)]
            s_row = [
                persist.tile([1, U], dt.bfloat16 if k == 0 else dt.float32,
                             name=f"sr{k}")
                for k in range(2)
            ]

            def wht_q(k, q):
                wt_ps = ps_a.tile([P, 512], dt.float32, tag="pa", name="wt_ps")
                nc.tensor.matmul(
                    wt_ps,
                    w2r[:, k * F : (k + 1) * F],
                    xtr[:, q * 512 : (q + 1) * 512],
                    start=True,
                    stop=True,
                )
                if q % 2 == 0:
                    nc.scalar.copy(whT[k][:, q * 512 : (q + 1) * 512], wt_ps)
                else:
                    nc.vector.tensor_copy(whT[k][:, q * 512 : (q + 1) * 512], wt_ps)

            def s_mms(k):
                for sq in range(2):
                    s_ps = ps_a.tile([1, 512], dt.float32, tag="pa", name="s_ps")
                    nc.tensor.matmul(
                        s_ps,
                        avr[:, 2 * k : 2 * k + 1],
                        whT[k][:, sq * 512 : (sq + 1) * 512],
                        start=True,
                        stop=True,
                    )
                    nc.scalar.copy(s_row[k][:, sq * 512 : (sq + 1) * 512], s_ps)

            def bcast(row, bc):
                for q in range(2):
                    bc_ps = ps_a.tile([P, 512], dt.float32, tag="pa", name="bc_ps")
                    nc.tensor.matmul(
                        bc_ps,
                        ones_b,
                        row[:, q * 512 : (q + 1) * 512],
                        start=True,
                        stop=True,
                    )
                    nc.vector.tensor_copy(bc[:, q * 512 : (q + 1) * 512], bc_ps)

            # ordered for shortest critical chains: s/broadcast work (needs
            # only the xt i-range half) first, then d work (j-range half)
            wht_q(0, 0)
            wht_q(0, 1)
            s_mms(0)
            s_bc0 = persist.tile([P, U], dt.bfloat16)
            bcast(s_row[0], s_bc0)
            wht_q(1, 0)
            wht_q(1, 1)
            s_mms(1)
            es_row = persist.tile([1, U], dt.bfloat16)
            nc.scalar.activation(es_row, s_row[1], Act.Exp)
            eas_row = persist.tile([1, U], dt.bfloat16)
            nc.scalar.activation(eas_row, s_row[1], Act.Exp, scale=ALPHA)
            es_bc = persist.tile([P, U], dt.bfloat16)
            bcast(es_row, es_bc)
            eas_bc = persist.tile([P, U], dt.bfloat16)
            bcast(eas_row, eas_bc)

            # d-scores via associativity: u = W @ a (columns, via the
            # host-provided W^T), then d[j] = x[j] . u -- the 8 quad
            # matmuls share the xtr tile as stationary weights
            u_ps = ps_a.tile([P, 4], dt.float32, tag="pa", name="u_ps")
            for k in range(2):
                nc.tensor.matmul(
                    u_ps[:, 2 * k : 2 * k + 2],
                    w2tr[:, k * F : (k + 1) * F],
                    avr[:, 2 * k : 2 * k + 2],
                    start=True,
                    stop=True,
                )
            u_sb = persist.tile([P, 4], dt.float32r)
            nc.scalar.copy(u_sb, u_ps)
            dq = ps_a.tile([P, 4 * JT], dt.float32, tag="dp", name="dq")
            for v in range(JT):
                nc.tensor.matmul(
                    dq[:, 4 * v : 4 * v + 4],
                    xtr[:, U + v * P : U + (v + 1) * P],
                    u_sb,
                    start=True,
                    stop=True,
                )
            dq4 = dq.rearrange("p (n four) -> p n four", four=4)
            d_cols = [None, None]
            for k in range(2):
                dc = persist.tile([P, JT], dt.float32, name=f"dc{k}")
                nc.scalar.copy(dc, dq4[:, :, 2 * k + 1 : 2 * k + 2])
                d_cols[k] = dc

            r1_cols = persist.tile([P, JT], dt.float32)
            nc.scalar.activation(r1_cols, dq4[:, :, 3:4], Act.Exp,
                                 scale=ALPHA - 1.0)
            s_row1b = persist.tile([1, U], dt.bfloat16)
            nc.scalar.copy(s_row1b, s_row[1])
            s_bc1 = persist.tile([P, U], dt.bfloat16)
            bcast(s_row1b, s_bc1)

            den_all = persist.tile([P, JT, 2], dt.float32)
            rec_all = persist.tile([P, JT, 2], dt.float32)
            out_ps = [
                [
                    ps_out.tile([P, 512], dt.float32, name=f"ops{k}{hf}")
                    for hf in range(2)
                ]
                for k in range(2)
            ]

            # ---------------- att phase
            # e0/m1 depend only on scores -- produce them all up front so
            # the per-tile critical path after each adjw arrival is just
            # g -> rec -> whs -> matmul (GpSimd paces adjw; ACT and DVE
            # pre-fill while it is blocked by the SWDGE adj stream)
            e0s, m1s = [], []
            for v in range(JT):
                lr = lr_pool.tile([P, U], dt.float32, tag="lr")
                nc.scalar.activation(
                    lr,
                    s_bc0,
                    Act.Prelu,
                    bias=d_cols[0][:, v : v + 1],
                    scale=1.0,
                    alpha=ALPHA,
                )
                e0 = e_pool.tile([P, U], dt.bfloat16, tag="e0", bufs=JT)
                nc.scalar.activation(e0, lr, Act.Exp)
                e0s.append(e0)
                m1 = e_pool.tile([P, U], dt.bfloat16, tag="m1", bufs=JT)
                if v >= 4:
                    lr1 = lr_pool.tile([P, U], dt.float32, tag="lr1")
                    nc.scalar.activation(
                        lr1,
                        s_bc1,
                        Act.Prelu,
                        bias=d_cols[1][:, v : v + 1],
                        scale=1.0,
                        alpha=ALPHA,
                    )
                    nc.scalar.activation(m1, lr1, Act.Exp)
                else:
                    nc.vector.scalar_tensor_tensor(
                        out=m1,
                        in0=eas_bc,
                        scalar=r1_cols[:, v : v + 1],
                        in1=es_bc,
                        op0=Alu.mult,
                        op1=Alu.max,
                    )
                m1s.append(m1)

            for v in range(JT):
                if True:
                    sl = slice(v * U, (v + 1) * U)
                    adjw = adjw_pool.tile([P, U], dt.bfloat16)
                    nc.gpsimd.tensor_tensor(
                        out=adjw, in0=asb[:, sl], in1=wsb[:, sl], op=Alu.mult
                    )
                    for k, e in ((0, e0s[v]), (1, m1s[v])):
                        g = g_pool.tile([P, U], dt.bfloat16, tag=f"g{k}")
                        nc.vector.scalar_tensor_tensor(
                            out=g,
                            in0=e,
                            scalar=1.0,
                            in1=adjw,
                            op0=Alu.mult,
                            op1=Alu.mult,
                            accum_out=den_all[:, v, k : k + 1],
                        )
                        rc = rec_all[:, v, k : k + 1]
                        nc.vector.reciprocal(rc, den_all[:, v, k : k + 1])
                        wh_ps = ps_a.tile([P, F], dt.float32, tag="pa")
                        nc.tensor.matmul(
                            wh_ps,
                            xtr[:, U + v * P : U + (v + 1) * P],
                            w2r[:, k * F : (k + 1) * F],
                            start=True,
                            stop=True,
                        )
                        whs = whs_pool.tile([P, F], dt.bfloat16)
                        if k == 0:
                            nc.vector.tensor_scalar(
                                out=whs, in0=wh_ps, scalar1=rc, scalar2=None,
                                op0=Alu.mult,
                            )
                        else:
                            nc.scalar.mul(whs, wh_ps, rc)
                        for half in range(2):
                            nc.tensor.matmul(
                                out_ps[k][half],
                                whs,
                                g[:, half * 512 : (half + 1) * 512],
                                start=(v == 0),
                                stop=(v == JT - 1),
                            )

            # ---------------- tail: elu + store (transposed out, host fixes)
            for k in range(2):
                o_sb = persist.tile([P, U], dt.float32, name=f"osb{k}")
                for half in range(2):
                    hs = slice(half * 512, (half + 1) * 512)
                    ps = out_ps[k][half]
                    E = elu_pool.tile([P, 512], dt.bfloat16, tag="E")
                    nc.scalar.activation(E, ps, Act.Exp)
                    E1 = elu_pool.tile([P, 512], dt.bfloat16, tag="E1")
                    nc.vector.tensor_scalar(
                        out=E1, in0=E, scalar1=-1.0, scalar2=0.0, op0=Alu.add,
                        op1=Alu.min,
                    )
                    nc.vector.scalar_tensor_tensor(
                        out=o_sb[:, hs],
                        in0=ps,
                        scalar=0.0,
                        in1=E1,
                        op0=Alu.max,
                        op1=Alu.add,
                    )
                    nc.sync.dma_start(out=outh[k, :, hs], in_=o_sb[:, hs])

    nc.compile()
    return nc


def kernel(x, weights, W, a, adj):
    global LAST_EXEC_NS
    from concourse.bass_utils import run_bass_kernel_spmd

    x = np.asarray(x, dtype=np.float32)
    weights = np.asarray(weights, dtype=np.float32)
    W = np.asarray(W, dtype=np.float32)
    a = np.asarray(a, dtype=np.float32)
    adj = np.asarray(adj, dtype=np.int32)

    with _BUILD_LOCK:
        if "nc" not in _CACHE:
            _CACHE["nc"] = _build_program()
    nc = _CACHE["nc"]

    def pack(m):
        # [1024, 1024] -> [128, 8*1024] row-tile packed
        return np.ascontiguousarray(
            m.reshape(JT, P, U).transpose(1, 0, 2).reshape(P, JT * U)
        )

    in_maps = []
    for c in range(8):
        b, blk, hp = c // 4, (c // 2) % 2, c % 2
        h0 = 2 * hp
        if blk == 0:  # block A: i in [0,U), j = U+v -> adjT[v,u], w natural
            adjT = adj[b, :U, U:].T
            wmat = weights[b]
            xtc = x[b].T
        else:  # block B: i = U+v, j = u -> adjT[u,v], w transposed
            adjT = adj[b, U:, :U].T
            wmat = weights[b].T
            xtc = np.concatenate([x[b, U:].T, x[b, :U].T], axis=1)
        in_maps.append(
            {
                "adjt": pack(adjT),
                "wq": pack(wmat),
                "xt": np.ascontiguousarray(xtc),
                "w2": np.ascontiguousarray(
                    np.concatenate([W[h0], W[h0 + 1]], axis=1)
                ),
                "w2t": np.ascontiguousarray(
                    np.concatenate([W[h0].T, W[h0 + 1].T], axis=1)
                ),
                "av": np.ascontiguousarray(
                    np.stack(
                        [a[h0, :F, 0], a[h0, F:, 0], a[h0 + 1, :F, 0],
                         a[h0 + 1, F:, 0]],
                        axis=1,
                    )
                ),
            }
        )

    res = run_bass_kernel_spmd(nc, in_maps, core_ids=list(range(8)), trace=TRACE)
    if res.exec_time_ns is not None:
        LAST_EXEC_NS = res.exec_time_ns

    out = np.empty((B, N, H * F), dtype=np.float32)
    for c in range(8):
        b, blk, hp = c // 4, (c // 2) % 2, c % 2
        h0 = 2 * hp
        rows = slice(0, U) if blk == 0 else slice(U, N)
        o = res.results[c]["outh"]  # [2, 128, 1024]
        for k in range(2):
            out[b, rows, (h0 + k) * F : (h0 + k + 1) * F] = o[k].T
    return out


# revision 28
# speedup vs baseline: 1.0760x; 1.0760x over previous
"""GAT message-passing kernel for Trainium2, 8 NeuronCores.

Problem (see harness reference): for each head h:
    Wh   = x @ W[h]                                  [B,N,F]
    e    = leaky_relu((Wh@a_src)[:,:,None] + (Wh@a_dst)[:,None,:], 0.2)
    att  = exp(where(adj>0, e, -9e15)) * big_w        [B,N,N]
    att /= clip(sum(att, axis=1), 1e-12)              (column L1 norm)
    out_h = elu(att @ Wh)
    out   = concat over heads                         [B,N,H*F]

big_w is bipartite: nonzero only on blocks (i<U, j>=U) [= weights.T] and
(i>=U, j<U) [= weights]. So att has only two 1024x1024 nonzero blocks.

Sharding: core c -> (b, block, head-pair) with b = c//4, blk = (c//2)%2,
hp = c%2.  Each core handles ONE bipartite block (its 1024 destination
rows i and 1024 source columns j) for TWO heads -> denominators are
core-local (each att column lives inside one block) and each core owns
1024 full output rows for its 2 heads.  No collectives, uniform SPMD.

All math runs in the transposed [j, i] layout.  The host pre-arranges
each core's shards so the device does ZERO transposes:
  - adjt: the core's adj block, transposed to [j, i] and row-tile packed
    to [128, 8*1024] (partition p, tile t, col i  <- adjT[t*128+p, i])
  - wq:   matching w values in the same [j, i] packed layout
  - xt:   x[b].T with columns ordered [i-range | j-range]
  - w2:   [128, 256] = W[h0] | W[h1],  av: [128,4] = a_src/a_dst pairs
  - w2t:  [128, 256] = W[h0].T | W[h1].T, for d = x @ (W a_dst)
Per-column exp factor cancellation: with z = s_i + d_j,
  exp(lrelu(z)) = max(e^z, e^az) = e^{d_j} * max(es_i, r_j * eas_i)
  (es = e^s, eas = e^{a s}, r = e^{(a-1)d}).  The e^{d_j} row factor
cancels against the denominator, so per head-tile the attention needs
just: m = (eas*r) max es (one DVE stt) and G = m*adjw with fused
row-sum -> den (one stt with accum_out).  Engine balance: head 0 and
half of head 1's tiles ride ACT (Prelu then Exp, bias=d column); the
rest use the DVE stt; adjw = adj*w builds on GpSimd (the only legal
Pool tensor_tensor: mult).  adj loads as a casting SWDGE DMA
(int32 -> bf16); x then w stream on the SP HWDGE ring (the ACT ring is
~3x slower -- params only); partition broadcasts are PE rank-1 outer
products (ones x row) because GpSimd blocks while SWDGE drains.
Scores use associativity: u = W @ a_dst via host-fed W^T, then
d[j] = x[j] . u -- 8 quad matmuls sharing the xtr stationary tile.
Output is accumulated transposed: outT[f,i] += whs[j,f]^T @ G[j,i]
with whs = Wh[j]/den[j] (Wh tiles matmul'd on demand, scaled from
PSUM), so matmuls are 512-wide; host un-transposes at gather.
elu(x) = max(x,0) + min(exp(x)-1, 0) with bf16 exp, stores per half.
"""

import threading
import numpy as np

B, N, FIN, F, H, U = 2, 2048, 128, 128, 4, 1024
P = 128
JT = U // P            # 8 tiles over the block's j axis
ALPHA = 0.2
CH = 2                 # v-tiles per DMA chunk (1MB chunks)
NCHUNK = JT // CH

TRACE = False          # set by test.py for profiling runs
LAST_EXEC_NS = None    # exec_time_ns of the last traced run
_BUILD_LOCK = threading.Lock()
_CACHE = {}


def _build_program():
    from concourse import bacc
    import concourse.mybir as mybir
    import concourse.tile as tile

    dt = mybir.dt
    Alu = mybir.AluOpType
    Act = mybir.ActivationFunctionType

    nc = bacc.Bacc("TRN2", target_bir_lowering=False, debug=False, num_devices=8,
                   num_swdge_queues=2)

    adjt = nc.dram_tensor("adjt", [P, JT * U], dt.int32, kind="ExternalInput")
    wq = nc.dram_tensor("wq", [P, JT * U], dt.float32, kind="ExternalInput")
    xt = nc.dram_tensor("xt", [P, N], dt.float32r, kind="ExternalInput")
    w2 = nc.dram_tensor("w2", [P, 2 * F], dt.float32r, kind="ExternalInput")
    av = nc.dram_tensor("av", [P, 4], dt.float32r, kind="ExternalInput")
    w2t = nc.dram_tensor("w2t", [P, 2 * F], dt.float32r, kind="ExternalInput")
    outh = nc.dram_tensor("outh", [2, P, U], dt.float32, kind="ExternalOutput")

    with tile.TileContext(nc) as tc:
        with (
            tc.tile_pool(name="persist", bufs=1) as persist,
            tc.tile_pool(name="adj_ch", bufs=3) as adj_pool,
            tc.tile_pool(name="w_ch", bufs=3) as w_pool,
            tc.tile_pool(name="adjw", bufs=3) as adjw_pool,
            tc.tile_pool(name="lr", bufs=2) as lr_pool,
            tc.tile_pool(name="ee", bufs=4) as e_pool,
            tc.tile_pool(name="gg", bufs=4) as g_pool,
            tc.tile_pool(name="whs", bufs=4) as whs_pool,
            tc.tile_pool(name="elu", bufs=4) as elu_pool,
            tc.tile_pool(name="ps_out", bufs=1, space="PSUM") as ps_out,
            tc.tile_pool(name="ps_a", bufs=2, space="PSUM") as ps_a,
        ):
            # ---------------- phase 0: params, xT, whT, scores
            w2r = persist.tile([P, 2 * F], dt.float32r)
            nc.scalar.dma_start(out=w2r, in_=w2[:, :])
            avr = persist.tile([P, 4], dt.float32r)
            nc.scalar.dma_start(out=avr, in_=av[:, :])
            w2tr = persist.tile([P, 2 * F], dt.float32r)
            nc.scalar.dma_start(out=w2tr, in_=w2t[:, :])
            xtr = persist.tile([P, N], dt.float32r)
            nc.sync.dma_start(out=xtr[:, 0:U], in_=xt[:, 0:U])
            nc.sync.dma_start(out=xtr[:, U:N], in_=xt[:, U:N])

            # bulk streams, issued up-front on otherwise-idle queues:
            # w on the SP HWDGE ring, adj via casting SWDGE (int32 -> bf16).
            # Subtile deps let per-v-tile consumers start as slices land.
            wsb = persist.tile([P, JT * U], dt.float32)
            asb = persist.tile([P, JT * U], dt.bfloat16)
            # w follows xt on the SP ring; small leading chunks so
            # adjw[0] unblocks early
            for lo, hi in ((0, 1), (1, 2), (2, 5), (5, 8)):
                sl = slice(lo * U, hi * U)
                nc.sync.dma_start(out=wsb[:, sl], in_=wq[:, sl])
            for lo in range(JT):
                sl = slice(lo * U, (lo + 1) * U)
                nc.gpsimd.dma_start(out=asb[:, sl], in_=adjt[:, sl])

            ones_b = persist.tile([1, P], dt.bfloat16)
            nc.vector.memset(ones_b, 1.0)

            whT = [persist.tile([P, N], dt.float32r, name=f"whT{k}") for k in range(2)]
            s_row = [
                persist.tile([1, U], dt.bfloat16 if k == 0 else dt.float32,
                             name=f"sr{k}")
                for k in range(2)
            ]

            def wht_q(k, q):
                wt_ps = ps_a.tile([P, 512], dt.float32, tag="pa", name="wt_ps")
                nc.tensor.matmul(
                    wt_ps,
                    w2r[:, k * F : (k + 1) * F],
                    xtr[:, q * 512 : (q + 1) * 512],
                    start=True,
                    stop=True,
                )
                if q % 2 == 0:
                    nc.scalar.copy(whT[k][:, q * 512 : (q + 1) * 512], wt_ps)
                else:
                    nc.vector.tensor_copy(whT[k][:, q * 512 : (q + 1) * 512], wt_ps)

            def s_mms(k):
                for sq in range(2):
                    s_ps = ps_a.tile([1, 512], dt.float32, tag="pa", name="s_ps")
                    nc.tensor.matmul(
                        s_ps,
                        avr[:, 2 * k : 2 * k + 1],
                        whT[k][:, sq * 512 : (sq + 1) * 512],
                        start=True,
                        stop=True,
                    )
                    nc.scalar.copy(s_row[k][:, sq * 512 : (sq + 1) * 512], s_ps)

            def bcast(row, bc):
                for q in range(2):
                    bc_ps = ps_a.tile([P, 512], dt.float32, tag="pa", name="bc_ps")
                    nc.tensor.matmul(
                        bc_ps,
                        ones_b,
                        row[:, q * 512 : (q + 1) * 512],
                        start=True,
                        stop=True,
                    )
                    nc.vector.tensor_copy(bc[:, q * 512 : (q + 1) * 512], bc_ps)

            # ordered for shortest critical chains: s/broadcast work (needs
            # only the xt i-range half) first, then d work (j-range half)
            wht_q(0, 0)
            wht_q(0, 1)
            s_mms(0)
            s_bc0 = persist.tile([P, U], dt.bfloat16)
            bcast(s_row[0], s_bc0)
            wht_q(1, 0)
            wht_q(1, 1)
            s_mms(1)
            es_row = persist.tile([1, U], dt.bfloat16)
            nc.scalar.activation(es_row, s_row[1], Act.Exp)
            eas_row = persist.tile([1, U], dt.bfloat16)
            nc.scalar.activation(eas_row, s_row[1], Act.Exp, scale=ALPHA)
            es_bc = persist.tile([P, U], dt.bfloat16)
            bcast(es_row, es_bc)
            eas_bc = persist.tile([P, U], dt.bfloat16)
            bcast(eas_row, eas_bc)

            # d-scores via associativity: u = W @ a (columns, via the
            # host-provided W^T), then d[j] = x[j] . u -- the 8 quad
            # matmuls share the xtr tile as stationary weights
            u_ps = ps_a.tile([P, 4], dt.float32, tag="pa", name="u_ps")
            for k in range(2):
                nc.tensor.matmul(
                    u_ps[:, 2 * k : 2 * k + 2],
                    w2tr[:, k * F : (k + 1) * F],
                    avr[:, 2 * k : 2 * k + 2],
                    start=True,
                    stop=True,
                )
            u_sb = persist.tile([P, 4], dt.float32r)
            nc.scalar.copy(u_sb, u_ps)
            dq = ps_a.tile([P, 4 * JT], dt.float32, tag="dp", name="dq")
            for v in range(JT):
                nc.tensor.matmul(
                    dq[:, 4 * v : 4 * v + 4],
                    xtr[:, U + v * P : U + (v + 1) * P],
                    u_sb,
                    start=True,
                    stop=True,
                )
            dq4 = dq.rearrange("p (n four) -> p n four", four=4)
            d_cols = [None, None]
            for k in range(2):
                dc = persist.tile([P, JT], dt.float32, name=f"dc{k}")
                nc.scalar.copy(dc, dq4[:, :, 2 * k + 1 : 2 * k + 2])
                d_cols[k] = dc

            r1_cols = persist.tile([P, JT], dt.float32)
            nc.scalar.activation(r1_cols, dq4[:, :, 3:4], Act.Exp,
                                 scale=ALPHA - 1.0)
            s_row1b = persist.tile([1, U], dt.bfloat16)
            nc.scalar.copy(s_row1b, s_row[1])
            s_bc1 = persist.tile([P, U], dt.bfloat16)
            bcast(s_row1b, s_bc1)

            den_all = persist.tile([P, JT, 2], dt.float32)
            rec_all = persist.tile([P, JT, 2], dt.float32)
            out_ps = [
                [
                    ps_out.tile([P, 512], dt.float32, name=f"ops{k}{hf}")
                    for hf in range(2)
                ]
                for k in range(2)
            ]

            # ---------------- att phase: one iteration per v-tile
            for v in range(JT):
                if True:
                    sl = slice(v * U, (v + 1) * U)
                    adjw = adjw_pool.tile([P, U], dt.bfloat16)
                    nc.gpsimd.tensor_tensor(
                        out=adjw, in0=asb[:, sl], in1=wsb[:, sl], op=Alu.mult
                    )
                    # head 0: ACT Prelu + Exp
                    lr = lr_pool.tile([P, U], dt.float32)
                    nc.scalar.activation(
                        lr,
                        s_bc0,
                        Act.Prelu,
                        bias=d_cols[0][:, v : v + 1],
                        scale=1.0,
                        alpha=ALPHA,
                    )
                    e0 = e_pool.tile([P, U], dt.bfloat16, tag="e0")
                    nc.scalar.activation(e0, lr, Act.Exp)
                    # head 1: last tiles ride ACT (it drains early),
                    # mid tiles lean on GpSimd for the max, the rest use
                    # the DVE max-of-exponentials trick
                    m1 = e_pool.tile([P, U], dt.bfloat16, tag="m1")
                    if v >= 4:
                        lr1 = lr_pool.tile([P, U], dt.float32, tag="lr1")
                        nc.scalar.activation(
                            lr1,
                            s_bc1,
                            Act.Prelu,
                            bias=d_cols[1][:, v : v + 1],
                            scale=1.0,
                            alpha=ALPHA,
                        )
                        nc.scalar.activation(m1, lr1, Act.Exp)
                    else:
                        nc.vector.scalar_tensor_tensor(
                            out=m1,
                            in0=eas_bc,
                            scalar=r1_cols[:, v : v + 1],
                            in1=es_bc,
                            op0=Alu.mult,
                            op1=Alu.max,
                        )
                    for k, e in ((0, e0), (1, m1)):
                        g = g_pool.tile([P, U], dt.bfloat16, tag=f"g{k}")
                        nc.vector.scalar_tensor_tensor(
                            out=g,
                            in0=e,
                            scalar=1.0,
                            in1=adjw,
                            op0=Alu.mult,
                            op1=Alu.mult,
                            accum_out=den_all[:, v, k : k + 1],
                        )
                        rc = rec_all[:, v, k : k + 1]
                        nc.vector.reciprocal(rc, den_all[:, v, k : k + 1])
                        wh_ps = ps_a.tile([P, F], dt.float32, tag="pa")
                        nc.tensor.matmul(
                            wh_ps,
                            xtr[:, U + v * P : U + (v + 1) * P],
                            w2r[:, k * F : (k + 1) * F],
                            start=True,
                            stop=True,
                        )
                        whs = whs_pool.tile([P, F], dt.bfloat16)
                        if k == 0:
                            nc.vector.tensor_scalar(
                                out=whs, in0=wh_ps, scalar1=rc, scalar2=None,
                                op0=Alu.mult,
                            )
                        else:
                            nc.scalar.mul(whs, wh_ps, rc)
                        for half in range(2):
                            nc.tensor.matmul(
                                out_ps[k][half],
                                whs,
                                g[:, half * 512 : (half + 1) * 512],
                                start=(v == 0),
                                stop=(v == JT - 1),
                            )

            # ---------------- tail: elu + store (transposed out, host fixes)
            for k in range(2):
                o_sb = persist.tile([P, U], dt.float32, name=f"osb{k}")
                for half in range(2):
                    hs = slice(half * 512, (half + 1) * 512)
                    ps = out_ps[k][half]
                    E = elu_pool.tile([P, 512], dt.bfloat16, tag="E")
                    nc.scalar.activation(E, ps, Act.Exp)
                    E1 = elu_pool.tile([P, 512], dt.bfloat16, tag="E1")
                    nc.vector.tensor_scalar(
                        out=E1, in0=E, scalar1=-1.0, scalar2=0.0, op0=Alu.add,
                        op1=Alu.min,
                    )
                    nc.vector.scalar_tensor_tensor(
                        out=o_sb[:, hs],
                        in0=ps,
                        scalar=0.0,
                        in1=E1,
                        op0=Alu.max,
                        op1=Alu.add,
                    )
                    nc.sync.dma_start(out=outh[k, :, hs], in_=o_sb[:, hs])

    nc.compile()
    return nc


def kernel(x, weights, W, a, adj):
    global LAST_EXEC_NS
    from concourse.bass_utils import run_bass_kernel_spmd

    x = np.asarray(x, dtype=np.float32)
    weights = np.asarray(weights, dtype=np.float32)
    W = np.asarray(W, dtype=np.float32)
    a = np.asarray(a, dtype=np.float32)
    adj = np.asarray(adj, dtype=np.int32)

    with _BUILD_LOCK:
        if "nc" not in _CACHE:
            _CACHE["nc"] = _build_program()
    nc = _CACHE["nc"]

    def pack(m):
        # [1024, 1024] -> [128, 8*1024] row-tile packed
        return np.ascontiguousarray(
            m.reshape(JT, P, U).transpose(1, 0, 2).reshape(P, JT * U)
        )

    in_maps = []
    for c in range(8):
        b, blk, hp = c // 4, (c // 2) % 2, c % 2
        h0 = 2 * hp
        if blk == 0:  # block A: i in [0,U), j = U+v -> adjT[v,u], w natural
            adjT = adj[b, :U, U:].T
            wmat = weights[b]
            xtc = x[b].T
        else:  # block B: i = U+v, j = u -> adjT[u,v], w transposed
            adjT = adj[b, U:, :U].T
            wmat = weights[b].T
            xtc = np.concatenate([x[b, U:].T, x[b, :U].T], axis=1)
        in_maps.append(
            {
                "adjt": pack(adjT),
                "wq": pack(wmat),
                "xt": np.ascontiguousarray(xtc),
                "w2": np.ascontiguousarray(
                    np.concatenate([W[h0], W[h0 + 1]], axis=1)
                ),
                "w2t": np.ascontiguousarray(
                    np.concatenate([W[h0].T, W[h0 + 1].T], axis=1)
                ),
                "av": np.ascontiguousarray(
                    np.stack(
                        [a[h0, :F, 0], a[h0, F:, 0], a[h0 + 1, :F, 0],
                         a[h0 + 1, F:, 0]],
                        axis=1,
                    )
                ),
            }
        )

    res = run_bass_kernel_spmd(nc, in_maps, core_ids=list(range(8)), trace=TRACE)
    if res.exec_time_ns is not None:
        LAST_EXEC_NS = res.exec_time_ns

    out = np.empty((B, N, H * F), dtype=np.float32)
    for c in range(8):
        b, blk, hp = c // 4, (c // 2) % 2, c % 2
        h0 = 2 * hp
        rows = slice(0, U) if blk == 0 else slice(U, N)
        o = res.results[c]["outh"]  # [2, 128, 1024]
        for k in range(2):
            out[b, rows, (h0 + k) * F : (h0 + k + 1) * F] = o[k].T
    return out
